# revision 9
# baseline (speedup 1.0000x reference)
import sys

if "/opt/trn_rl_repo" not in sys.path:
    sys.path.insert(0, "/opt/trn_rl_repo")

from contextlib import ExitStack

import numpy as np
import concourse.bass as bass
import concourse.mybir as mybir
from concourse.bass_utils import run_bass_kernel_spmd

# Problem: loss = sum_b ||cos(2pi(output_b-0.5))|| * ||cos(2pi(target_b-0.5))||
# for output/target of shape [4096, 4096] f32, values in [0, 1).
#
# Math used on device: with theta = 2pi*x - pi (in [-pi, pi), where the Sin
# LUT is accurate), s = sin(theta) and cos^2(2pi*(x-0.5)) = cos^2(theta)
# = 1 - s^2. So per-row sumsq = N - sum(s^2). The device returns per-row
# sum(s^2); sqrt/product/final sum happen on host in float64.

B, N = 4096, 4096
N_CORES = 8
ROWS_PER_CORE = B // N_CORES  # 512
P = 128
TILES_PER_TENSOR = ROWS_PER_CORE // P  # 4
N_TILES = 2 * TILES_PER_TENSOR  # 8 (output tiles then target tiles)
N_BUF = 4
TWO_PI = 2.0 * np.pi

_CACHE = {}


def _build():
    nc = bass.Bass()
    o_ext = nc.declare_dram_parameter(
        "output", [ROWS_PER_CORE, N], mybir.dt.float32, isOutput=False
    )
    t_ext = nc.declare_dram_parameter(
        "target", [ROWS_PER_CORE, N], mybir.dt.float32, isOutput=False
    )
    acc_ext = nc.declare_dram_parameter(
        "acc", [P, N_TILES], mybir.dt.float32, isOutput=True
    )

    tile_aps = [
        ext[i * P : (i + 1) * P, :]
        for ext in (o_ext, t_ext)
        for i in range(TILES_PER_TENSOR)
    ]

    bias_t = nc.alloc_sbuf_tensor("const_neg_pi", [P, 1], mybir.dt.float32)
    nc.gpsimd.memset(bias_t.ap(), float(-np.pi))
    nc.all_engine_barrier()

    with (
        ExitStack() as ctx,
        nc.semaphore("dma_sem") as dma_sem,
        nc.semaphore("act_sem") as act_sem,
        nc.semaphore("dve_sem") as dve_sem,
        nc.semaphore("out_sem") as out_sem,
        nc.Block() as block,
    ):
        in_bufs = [
            ctx.enter_context(nc.sbuf_tensor(f"in_buf{i}", [P, N], mybir.dt.float32))
            for i in range(N_BUF)
        ]
        res_bufs = [
            ctx.enter_context(nc.sbuf_tensor(f"res_buf{i}", [P, N], mybir.dt.float32))
            for i in range(2)
        ]
        scratch = ctx.enter_context(
            nc.sbuf_tensor("scratch", [P, 1], mybir.dt.float32)
        )
        acc = ctx.enter_context(
            nc.sbuf_tensor("acc_sb", [P, N_TILES], mybir.dt.float32)
        )

        @block.sync
        def _(sync):
            for i, dram_ap in enumerate(tile_aps):
                if i >= N_BUF:
                    # Sin of tile i-N_BUF must be done reading this buffer.
                    sync.wait_ge(act_sem, i - N_BUF + 1)
                sync.dma_start(out=in_bufs[i % N_BUF][:], in_=dram_ap).then_inc(
                    dma_sem, 16
                )
            sync.wait_ge(dve_sem, N_TILES)
            sync.dma_start(out=acc_ext[:], in_=acc[:]).then_inc(out_sem, 16)
            sync.wait_ge(out_sem, 16)

        @block.scalar
        def _(scalar):
            for i in range(N_TILES):
                scalar.wait_ge(dma_sem, 16 * (i + 1))
                if i >= 2:
                    # TTR of tile i-2 must be done reading res_bufs[i % 2].
                    scalar.wait_ge(dve_sem, i - 1)
                scalar.activation(
                    res_bufs[i % 2][:],
                    in_bufs[i % N_BUF][:],
                    mybir.ActivationFunctionType.Sin,
                    bias=bias_t.ap(),
                    scale=TWO_PI,
                ).then_inc(act_sem, 1)

        @block.vector
        def _(vector):
            for i in range(N_TILES):
                vector.wait_ge(act_sem, i + 1)
                vector.scalar_tensor_tensor(
                    out=scratch[:].broadcast_to([P, N]),
                    in0=res_bufs[i % 2][:],
                    scalar=1.0,
                    in1=res_bufs[i % 2][:],
                    op0=mybir.AluOpType.mult,
                    op1=mybir.AluOpType.mult,
                    accum_out=acc[:, i : i + 1],
                ).then_inc(dve_sem, 1)

    return nc


def _get_nc():
    if "nc" not in _CACHE:
        _CACHE["nc"] = _build()
    return _CACHE["nc"]


def kernel(output: np.ndarray, target: np.ndarray) -> np.ndarray:
    output = np.ascontiguousarray(output, dtype=np.float32)
    target = np.ascontiguousarray(target, dtype=np.float32)
    nc = _get_nc()
    in_maps = [
        {
            "output": output[c * ROWS_PER_CORE : (c + 1) * ROWS_PER_CORE],
            "target": target[c * ROWS_PER_CORE : (c + 1) * ROWS_PER_CORE],
        }
        for c in range(N_CORES)
    ]
    results = run_bass_kernel_spmd(nc, in_maps, core_ids=list(range(N_CORES))).results

    total = 0.0
    for c in range(N_CORES):
        acc = results[c]["acc"].astype(np.float64)  # [P, N_TILES] = sum(sin^2) per row
        sumsq = np.maximum(float(N) - acc, 0.0)
        so = sumsq[:, :TILES_PER_TENSOR]
        st = sumsq[:, TILES_PER_TENSOR:]
        total += np.sqrt(so * st).sum()
    return np.array(total, dtype=np.float32)


# revision 10
# speedup vs baseline: 1.0554x; 1.0554x over previous
import sys

if "/opt/trn_rl_repo" not in sys.path:
    sys.path.insert(0, "/opt/trn_rl_repo")

from contextlib import ExitStack

import numpy as np
import concourse.bass as bass
import concourse.mybir as mybir
from concourse.bass_utils import run_bass_kernel_spmd

# Problem: loss = sum_b ||cos(2pi(output_b-0.5))|| * ||cos(2pi(target_b-0.5))||
# for output/target of shape [4096, 4096] f32, values in [0, 1).
#
# Math used on device: with theta = 2pi*x - pi (in [-pi, pi), where the Sin
# LUT is accurate), s = sin(theta) and cos^2(2pi*(x-0.5)) = cos^2(theta)
# = 1 - s^2. So per-row sumsq = N - sum(s^2). The device returns per-tile
# partial sum(s^2) per row; sqrt/product/final sum happen on host in float64.

B, N = 4096, 4096
N_CORES = 8
ROWS_PER_CORE = B // N_CORES  # 512
P = 128
ROW_BLOCKS = ROWS_PER_CORE // P  # 4
COL_SPLIT = 2  # split each row into this many free-dim chunks
FREE = N // COL_SPLIT  # 2048
TILES_PER_TENSOR = ROW_BLOCKS * COL_SPLIT  # 8
N_TILES = 2 * TILES_PER_TENSOR  # 16 (output tiles then target tiles)
N_BUF = 6
N_RES = 3
TWO_PI = 2.0 * np.pi

_CACHE = {}


def _build():
    nc = bass.Bass()
    o_ext = nc.declare_dram_parameter(
        "output", [ROWS_PER_CORE, N], mybir.dt.float32, isOutput=False
    )
    t_ext = nc.declare_dram_parameter(
        "target", [ROWS_PER_CORE, N], mybir.dt.float32, isOutput=False
    )
    acc_ext = nc.declare_dram_parameter(
        "acc", [P, N_TILES], mybir.dt.float32, isOutput=True
    )

    # tile (r, h) of a tensor = rows r*P..(r+1)*P, cols h*FREE..(h+1)*FREE
    tile_aps = [
        ext[r * P : (r + 1) * P, h * FREE : (h + 1) * FREE]
        for ext in (o_ext, t_ext)
        for r in range(ROW_BLOCKS)
        for h in range(COL_SPLIT)
    ]

    one_ap = nc.const_aps.tensor(1.0, (P, 1), mybir.dt.float32)

    with (
        ExitStack() as ctx,
        nc.semaphore("dma_sem") as dma_sem,
        nc.semaphore("act_sem") as act_sem,
        nc.semaphore("dve_sem") as dve_sem,
        nc.semaphore("out_sem") as out_sem,
        nc.Block() as block,
    ):
        in_bufs = [
            ctx.enter_context(
                nc.sbuf_tensor(f"in_buf{i}", [P, FREE], mybir.dt.float32)
            )
            for i in range(N_BUF)
        ]
        res_bufs = [
            ctx.enter_context(
                nc.sbuf_tensor(f"res_buf{i}", [P, FREE], mybir.dt.float32)
            )
            for i in range(N_RES)
        ]
        scratch = ctx.enter_context(
            nc.sbuf_tensor("scratch", [P, 1], mybir.dt.float32)
        )
        bias_t = ctx.enter_context(
            nc.sbuf_tensor("bias_neg_pi", [P, 1], mybir.dt.float32)
        )
        acc = ctx.enter_context(
            nc.sbuf_tensor("acc_sb", [P, N_TILES], mybir.dt.float32)
        )

        @block.sync
        def _(sync):
            for i, dram_ap in enumerate(tile_aps):
                if i >= N_BUF:
                    # Sin of tile i-N_BUF must be done reading this buffer.
                    sync.wait_ge(act_sem, i - N_BUF + 1)
                sync.dma_start(out=in_bufs[i % N_BUF][:], in_=dram_ap).then_inc(
                    dma_sem, 16
                )
            sync.wait_ge(dve_sem, N_TILES)
            sync.dma_start(out=acc_ext[:], in_=acc[:]).then_inc(out_sem, 16)
            sync.wait_ge(out_sem, 16)

        @block.scalar
        def _(scalar):
            # bias_t = -pi, produced on the consuming engine (no cross-engine
            # sync needed; the pre-registered const-1.0 AP is barrier-ready).
            scalar.mul(bias_t[:], one_ap, float(-np.pi))
            for i in range(N_TILES):
                scalar.wait_ge(dma_sem, 16 * (i + 1))
                if i >= N_RES:
                    # STT of tile i-N_RES must be done reading res_bufs[i%N_RES].
                    scalar.wait_ge(dve_sem, i - N_RES + 1)
                scalar.activation(
                    res_bufs[i % N_RES][:],
                    in_bufs[i % N_BUF][:],
                    mybir.ActivationFunctionType.Sin,
                    bias=bias_t[:],
                    scale=TWO_PI,
                ).then_inc(act_sem, 1)

        @block.vector
        def _(vector):
            for i in range(N_TILES):
                vector.wait_ge(act_sem, i + 1)
                vector.scalar_tensor_tensor(
                    out=scratch[:].broadcast_to([P, FREE]),
                    in0=res_bufs[i % N_RES][:],
                    scalar=1.0,
                    in1=res_bufs[i % N_RES][:],
                    op0=mybir.AluOpType.mult,
                    op1=mybir.AluOpType.mult,
                    accum_out=acc[:, i : i + 1],
                ).then_inc(dve_sem, 1)

    return nc


def _get_nc():
    if "nc" not in _CACHE:
        _CACHE["nc"] = _build()
    return _CACHE["nc"]


def kernel(output: np.ndarray, target: np.ndarray) -> np.ndarray:
    output = np.ascontiguousarray(output, dtype=np.float32)
    target = np.ascontiguousarray(target, dtype=np.float32)
    nc = _get_nc()
    in_maps = [
        {
            "output": output[c * ROWS_PER_CORE : (c + 1) * ROWS_PER_CORE],
            "target": target[c * ROWS_PER_CORE : (c + 1) * ROWS_PER_CORE],
        }
        for c in range(N_CORES)
    ]
    results = run_bass_kernel_spmd(nc, in_maps, core_ids=list(range(N_CORES))).results

    total = 0.0
    for c in range(N_CORES):
        acc = results[c]["acc"].astype(np.float64)  # [P, N_TILES]
        # per-row sum over the COL_SPLIT chunks of each tensor's row-block
        acc_o = acc[:, :TILES_PER_TENSOR].reshape(P, ROW_BLOCKS, COL_SPLIT).sum(axis=2)
        acc_t = acc[:, TILES_PER_TENSOR:].reshape(P, ROW_BLOCKS, COL_SPLIT).sum(axis=2)
        so = np.maximum(float(N) - acc_o, 0.0)
        st = np.maximum(float(N) - acc_t, 0.0)
        total += np.sqrt(so * st).sum()
    return np.array(total, dtype=np.float32)


# revision 11
# speedup vs baseline: 1.0665x; 1.0105x over previous
import sys

if "/opt/trn_rl_repo" not in sys.path:
    sys.path.insert(0, "/opt/trn_rl_repo")

from contextlib import ExitStack

import numpy as np
import concourse.bass as bass
import concourse.mybir as mybir
from concourse.bass_utils import run_bass_kernel_spmd

# Problem: loss = sum_b ||cos(2pi(output_b-0.5))|| * ||cos(2pi(target_b-0.5))||
# for output/target of shape [4096, 4096] f32, values in [0, 1).
#
# Math used on device: with theta = 2pi*x - pi (in [-pi, pi), where the Sin
# LUT is accurate), s = sin(theta) and cos^2(2pi*(x-0.5)) = cos^2(theta)
# = 1 - s^2. So per-row sumsq = N - sum(s^2). The device returns per-tile
# partial sum(s^2) per row; sqrt/product/final sum happen on host in float64.

B, N = 4096, 4096
N_CORES = 8
ROWS_PER_CORE = B // N_CORES  # 512
P = 128
ROW_BLOCKS = ROWS_PER_CORE // P  # 4
COL_SPLIT = 2  # split each row into this many free-dim chunks
FREE = N // COL_SPLIT  # 2048
TILES_PER_TENSOR = ROW_BLOCKS * COL_SPLIT  # 8
N_TILES = 2 * TILES_PER_TENSOR  # 16 (output tiles then target tiles)
N_BUF = 6
N_RES = 3
TWO_PI = 2.0 * np.pi

_CACHE = {}


def _build():
    nc = bass.Bass()
    o_ext = nc.declare_dram_parameter(
        "output", [ROWS_PER_CORE, N], mybir.dt.float32, isOutput=False
    )
    t_ext = nc.declare_dram_parameter(
        "target", [ROWS_PER_CORE, N], mybir.dt.float32, isOutput=False
    )
    acc_ext = nc.declare_dram_parameter(
        "acc", [P, N_TILES], mybir.dt.float32, isOutput=True
    )

    # tile (r, h) of a tensor = rows r*P..(r+1)*P, cols h*FREE..(h+1)*FREE
    tile_aps = [
        ext[r * P : (r + 1) * P, h * FREE : (h + 1) * FREE]
        for ext in (o_ext, t_ext)
        for r in range(ROW_BLOCKS)
        for h in range(COL_SPLIT)
    ]

    one_ap = nc.const_aps.tensor(1.0, (P, 1), mybir.dt.float32)

    with (
        ExitStack() as ctx,
        nc.semaphore("dma_sem") as dma_sem,
        nc.semaphore("act_sem") as act_sem,
        nc.semaphore("dve_sem") as dve_sem,
        nc.semaphore("out_sem") as out_sem,
        nc.Block(no_gpsimd_drain=True) as block,
    ):
        in_bufs = [
            ctx.enter_context(
                nc.sbuf_tensor(f"in_buf{i}", [P, FREE], mybir.dt.float32)
            )
            for i in range(N_BUF)
        ]
        res_bufs = [
            ctx.enter_context(
                nc.sbuf_tensor(f"res_buf{i}", [P, FREE], mybir.dt.float32)
            )
            for i in range(N_RES)
        ]
        scratch = ctx.enter_context(
            nc.sbuf_tensor("scratch", [P, 1], mybir.dt.float32)
        )
        bias_t = ctx.enter_context(
            nc.sbuf_tensor("bias_neg_pi", [P, 1], mybir.dt.float32)
        )
        acc = ctx.enter_context(
            nc.sbuf_tensor("acc_sb", [P, N_TILES], mybir.dt.float32)
        )

        @block.sync
        def _(sync):
            for i, dram_ap in enumerate(tile_aps):
                if i >= N_BUF:
                    # Sin of tile i-N_BUF must be done reading this buffer.
                    sync.wait_ge(act_sem, i - N_BUF + 1)
                sync.dma_start(out=in_bufs[i % N_BUF][:], in_=dram_ap).then_inc(
                    dma_sem, 16
                )
            sync.wait_ge(dve_sem, N_TILES)
            sync.dma_start(out=acc_ext[:], in_=acc[:]).then_inc(out_sem, 16)
            sync.wait_ge(out_sem, 16)

        @block.scalar
        def _(scalar):
            # bias_t = -pi, produced on the consuming engine (no cross-engine
            # sync needed; the pre-registered const-1.0 AP is barrier-ready).
            scalar.mul(bias_t[:], one_ap, float(-np.pi))
            for i in range(N_TILES):
                scalar.wait_ge(dma_sem, 16 * (i + 1))
                if i >= N_RES:
                    # STT of tile i-N_RES must be done reading res_bufs[i%N_RES].
                    scalar.wait_ge(dve_sem, i - N_RES + 1)
                scalar.activation(
                    res_bufs[i % N_RES][:],
                    in_bufs[i % N_BUF][:],
                    mybir.ActivationFunctionType.Sin,
                    bias=bias_t[:],
                    scale=TWO_PI,
                ).then_inc(act_sem, 1)

        @block.vector
        def _(vector):
            for i in range(N_TILES):
                vector.wait_ge(act_sem, i + 1)
                vector.scalar_tensor_tensor(
                    out=scratch[:].broadcast_to([P, FREE]),
                    in0=res_bufs[i % N_RES][:],
                    scalar=1.0,
                    in1=res_bufs[i % N_RES][:],
                    op0=mybir.AluOpType.mult,
                    op1=mybir.AluOpType.mult,
                    accum_out=acc[:, i : i + 1],
                ).then_inc(dve_sem, 1)

    return nc


def _get_nc():
    if "nc" not in _CACHE:
        _CACHE["nc"] = _build()
    return _CACHE["nc"]


def kernel(output: np.ndarray, target: np.ndarray) -> np.ndarray:
    output = np.ascontiguousarray(output, dtype=np.float32)
    target = np.ascontiguousarray(target, dtype=np.float32)
    nc = _get_nc()
    in_maps = [
        {
            "output": output[c * ROWS_PER_CORE : (c + 1) * ROWS_PER_CORE],
            "target": target[c * ROWS_PER_CORE : (c + 1) * ROWS_PER_CORE],
        }
        for c in range(N_CORES)
    ]
    results = run_bass_kernel_spmd(nc, in_maps, core_ids=list(range(N_CORES))).results

    total = 0.0
    for c in range(N_CORES):
        acc = results[c]["acc"].astype(np.float64)  # [P, N_TILES]
        # per-row sum over the COL_SPLIT chunks of each tensor's row-block
        acc_o = acc[:, :TILES_PER_TENSOR].reshape(P, ROW_BLOCKS, COL_SPLIT).sum(axis=2)
        acc_t = acc[:, TILES_PER_TENSOR:].reshape(P, ROW_BLOCKS, COL_SPLIT).sum(axis=2)
        so = np.maximum(float(N) - acc_o, 0.0)
        st = np.maximum(float(N) - acc_t, 0.0)
        total += np.sqrt(so * st).sum()
    return np.array(total, dtype=np.float32)
